# revision 46
# baseline (speedup 1.0000x reference)
"""Multi-head causal attention (B=4, S=2048, D=1024, H=16) on 8 TRN2 cores.

Sharding: data-parallel over batch (4) x tensor-parallel over heads (2 groups
of 8 heads). Core c handles batch c//2, head-group c%2. Each core computes
q/k/v projections for its 8 heads, causal flash-style attention, and a partial
output projection against its row-shard of Wp. Host sums the two partials per
batch and adds the bias terms (bp + bv @ Wp, which commute with the row-sum).

Key layout/scheduling choices (v3):
- x arrives pre-transposed, pre-tiled and pre-cast to bf16 from the host, as do
  all weights: DMA descriptors are 4-8KB contiguous runs (descriptor-rate is
  the startup bottleneck), and there are no on-chip transposes or casts.
- Scores are computed as S^T = kT^T qT with keys on partitions; the softmax
  denominator falls out of the AV matmul via a ones-column appended to V.
- Heads are processed in (even, odd) pairs living on partitions 0-63 / 64-127.
  The two K=64 S-matmuls of a pair are emitted back-to-back so the PE array
  row-tiles them (tile_position (0,0) / (64,0)) and runs them concurrently.
  One unit = (pair, key-block): S psum is [128, 2(parity), 512] so a single
  exp ACT covers both heads of the pair (amortizes the ~290ns ACT overhead).
- Diagonal 128-key blocks only stream the causally-live query range (q0 =
  128*(kb-4g)); exp is likewise restricted, and one [128, 2, 128] lower-tri
  mask handles the partial block. Fully masked regions are never written/read.
- exp/AV emission lags S emission by one unit (shared queue) so the scalar
  engine's exp pipelines with the tensor engine; qkv/proj work for other
  groups is woven into the exp-bound attention stretches as fill.
- Softmax denominators round-trip through DRAM for a 128-lane reciprocal; the
  final oT multiply is deferred by one head-pair so the DMA latency never
  blocks the (strict-FIFO) vector engine queue.
"""

import numpy as np

B, S, D, H = 4, 2048, 1024, 16
HD = D // H            # head_size = 64
HPC = 8                # heads per core
LCOL = HPC * HD        # 512 local columns
NSG = 4                # seq groups of 512
SG = S // NSG          # 512
NKB = S // 128         # 16 key blocks of 128

_CACHE = {}


def _build(debug_dump=False):
    import concourse.bass as bass
    import concourse.tile as tile
    from concourse import bacc, mybir

    f32 = mybir.dt.float32
    bf16 = mybir.dt.bfloat16

    nc = bacc.Bacc("TRN2", target_bir_lowering=False, debug=False)

    xtg_d = nc.dram_tensor("xtg", [NSG, 128, 8, SG], bf16, kind="ExternalInput")
    wq_d = nc.dram_tensor("wq", [128, 8, LCOL], bf16, kind="ExternalInput")
    wk_d = nc.dram_tensor("wk", [128, 8, LCOL], bf16, kind="ExternalInput")
    wv_d = nc.dram_tensor("wv", [128, 8, LCOL], bf16, kind="ExternalInput")
    wp_d = nc.dram_tensor("wp", [128, 4, D], bf16, kind="ExternalInput")
    bq_d = nc.dram_tensor("bq", [LCOL], f32, kind="ExternalInput")
    bk_d = nc.dram_tensor("bk", [LCOL], f32, kind="ExternalInput")
    tri_d = nc.dram_tensor("tri", [128, 2, 128], bf16, kind="ExternalInput")
    out_d = nc.dram_tensor("out", [S, D], bf16, kind="ExternalOutput")

    Exp = mybir.ActivationFunctionType.Exp

    with tile.TileContext(nc) as tc:
        with (
            tc.tile_pool(name="consts", bufs=1) as consts,
            tc.tile_pool(name="xtp", bufs=2) as xtp,
            tc.tile_pool(name="acts", bufs=1) as acts,
            tc.tile_pool(name="pp", bufs=6) as pp,
            tc.tile_pool(name="recp", bufs=6) as recp,
            tc.tile_pool(name="orp", bufs=8) as orp,
            tc.tile_pool(name="rp", bufs=6) as rp,
            tc.tile_pool(name="outp", bufs=4) as outp,
            tc.tile_pool(name="drp", bufs=2, space="DRAM") as drp,
            tc.tile_pool(name="ps_s", bufs=2, space="PSUM") as ps_s,
            tc.tile_pool(name="ps_o", bufs=2, space="PSUM") as ps_o,
            tc.tile_pool(name="ps_f", bufs=2, space="PSUM") as ps_f,
        ):
            # ---- weights (host already bf16 + pre-tiled; 2 DMAs each) -----
            def load_w(dram, shape, name, nsp=2):
                t = consts.tile(shape, bf16, name=name)
                w = shape[1] // nsp
                for i in range(nsp):
                    nc.sync.dma_start(out=t[:, i * w:(i + 1) * w, :],
                                      in_=dram.ap()[:, i * w:(i + 1) * w, :])
                return t



            # ---- persistent activations ----------------------------------
            qT = acts.tile([128, 4, S], bf16)      # [head-dim%128, pair, seq]
            kT = acts.tile([128, 4, S], bf16)
            oT = acts.tile([128, 4, S], bf16)
            v_ext = acts.tile([128, NKB, HPC, HD + 1], bf16)
            for h in range(HPC):                   # ones columns (denominator)
                nc.vector.memset(v_ext[:, :, h, HD:HD + 1], 1.0)

            def qkv_gen(g):
                """q/k/v projections for seq group g. Chunk order: q/k for
                pair 0, then v (all kbs), then q/k for pairs 1-3 so the
                attention of (g, pair 0) can start as early as possible."""
                xT = xtp.tile([128, 8, SG], bf16, name="xT", tag="xT")
                nc.sync.dma_start(out=xT[:, 0:4, :], in_=xtg_d.ap()[g][:, 0:4, :])
                nc.sync.dma_start(out=xT[:, 4:8, :], in_=xtg_d.ap()[g][:, 4:8, :])
                yield

                def qk_m(m):
                    for w_sb, b_sb, dstT in ((wq_sb, bq_sb, qT),
                                             (wk_sb, bk_sb, kT)):
                        pq = ps_f.tile([128, SG], f32, name="pq", tag="ps_f")
                        for dc in range(8):
                            nc.tensor.matmul(
                                pq, lhsT=w_sb[:, dc, 128 * m:128 * (m + 1)],
                                rhs=xT[:, dc, :], start=(dc == 0),
                                stop=(dc == 7))
                        nc.vector.tensor_scalar_add(
                            dstT[:, m, g * SG:(g + 1) * SG], pq,
                            b_sb[:, m:m + 1])
                        yield

                yield from qk_m(0)
                for s4 in range(4):
                    pv = ps_f.tile([128, LCOL], f32, name="pv", tag="ps_f")
                    for dc in range(8):
                        nc.tensor.matmul(
                            pv, lhsT=xT[:, dc, 128 * s4:128 * (s4 + 1)],
                            rhs=wv_sb[:, dc, :], start=(dc == 0), stop=(dc == 7))
                    kb = 4 * g + s4
                    nc.vector.tensor_copy(
                        v_ext[:, kb, :, 0:HD],
                        pv.rearrange("p (h e) -> p h e", e=HD))
                    yield
                for m in range(1, 4):
                    yield from qk_m(m)

            def proj_gen(g, s4s=(0, 1, 2, 3)):
                for s4 in s4s:
                    sb = 4 * g + s4
                    o_sb = outp.tile([128, 2, SG], bf16, name="o_sb", tag="o_sb")
                    for j in range(2):
                        ppr = ps_f.tile([128, SG], f32, name="ppr", tag="ps_f")
                        for c in range(4):
                            nc.tensor.matmul(
                                ppr, lhsT=oT[:, c, 128 * sb:128 * (sb + 1)],
                                rhs=wp_sb[:, c, j * SG:(j + 1) * SG],
                                start=(c == 0), stop=(c == 3))
                        nc.vector.tensor_copy(o_sb[:, j, :], ppr)
                        yield
                    nc.sync.dma_start(
                        out=out_d.ap()[128 * sb:128 * (sb + 1), :]
                        .rearrange("p (j n) -> p j n", j=2),
                        in_=o_sb)

            def attn_unit(ct, g, kb, pair_state, queue, tick):
                """One (head-pair, key-block) unit: emits the two row-tiled
                S matmuls now; queues exp+mask+AV for later."""
                nkb = 4 * g + 4
                q0 = max(0, 128 * kb - g * SG)
                pst = ps_s.tile([128, 2, SG], f32, name="pst", tag="ps_s")
                for parity in (0, 1):
                    po = slice(64 * parity, 64 * parity + 64)
                    nc.tensor.matmul(
                        pst[:, parity, q0:SG],
                        lhsT=kT[po, ct, 128 * kb:128 * (kb + 1)],
                        rhs=qT[po, ct, g * SG + q0:(g + 1) * SG],
                        start=True, stop=True)
                tick()

                def exp_av():
                    if kb == 0:
                        pair_state[0] = ps_o.tile([HD + 1, SG], f32,
                                                  name="po0", tag="ps_o")
                        pair_state[1] = ps_o.tile([HD + 1, SG], f32,
                                                  name="po1", tag="ps_o")
                    p_sb = pp.tile([128, 2, SG], bf16, name="p_sb", tag="p_sb")
                    nc.scalar.activation(p_sb[:, :, q0:SG], pst[:, :, q0:SG],
                                         Exp, scale=0.125)
                    if kb >= 4 * g:    # diagonal block: lower-tri mask
                        nc.vector.tensor_mul(
                            p_sb[:, :, q0:q0 + 128], p_sb[:, :, q0:q0 + 128],
                            tri_sb)
                    for parity in (0, 1):
                        h = 2 * ct + parity
                        nc.tensor.matmul(
                            pair_state[parity][:, q0:SG],
                            lhsT=v_ext[:, kb, h, :],
                            rhs=p_sb[:, parity, q0:SG],
                            start=(kb == 0), stop=(kb == nkb - 1))

                queue.append(exp_av)
                while len(queue) > 1:
                    queue.pop(0)()

            def attn_unit_m(ct, g, pair_state, queue, tick):
                """Merged unit for the last two diagonal key-blocks: their
                causally-live query ranges (256 + 128 cols) share one PSUM
                bank pair, so a single exp ACT covers both (saves ~290ns of
                ACT overhead per pair). kb_a's range q[256:512) is stored at
                cols [0:256); kb_b's q[384:512) at cols [256:384)."""
                nkb = 4 * g + 4
                kb_a, kb_b = 4 * g + 2, 4 * g + 3
                pst = ps_s.tile([128, 2, SG], f32, name="pst", tag="ps_s")
                for kb, c0, q0 in ((kb_a, 0, 256), (kb_b, 256, 384)):
                    n = SG - q0
                    for parity in (0, 1):
                        po = slice(64 * parity, 64 * parity + 64)
                        nc.tensor.matmul(
                            pst[:, parity, c0:c0 + n],
                            lhsT=kT[po, ct, 128 * kb:128 * (kb + 1)],
                            rhs=qT[po, ct, g * SG + q0:(g + 1) * SG],
                            start=True, stop=True)
                tick()

                def exp_av():
                    p_sb = pp.tile([128, 2, SG], bf16, name="p_sb", tag="p_sb")
                    nc.scalar.activation(p_sb[:, :, 0:384], pst[:, :, 0:384],
                                         Exp, scale=0.125)
                    nc.vector.tensor_mul(p_sb[:, :, 0:128],
                                         p_sb[:, :, 0:128], tri_sb)
                    nc.vector.tensor_mul(p_sb[:, :, 256:384],
                                         p_sb[:, :, 256:384], tri_sb)
                    for parity in (0, 1):
                        h = 2 * ct + parity
                        nc.tensor.matmul(
                            pair_state[parity][:, 256:SG],
                            lhsT=v_ext[:, kb_a, h, :],
                            rhs=p_sb[:, parity, 0:256],
                            start=False, stop=False)
                        nc.tensor.matmul(
                            pair_state[parity][:, 384:SG],
                            lhsT=v_ext[:, kb_b, h, :],
                            rhs=p_sb[:, parity, 256:384],
                            start=False, stop=(kb_b == nkb - 1))

                queue.append(exp_av)
                while len(queue) > 1:
                    queue.pop(0)()

            # Normalization runs as a 3-stage pipeline, each stage deferred
            # by one head-pair so no DVE op ever waits on a DMA round-trip
            # at the head of the (strict FIFO) vector queue.
            def make_normA(ct, g, pair_state, parity, handoff, lnexp=False):
                def normA():
                    psum_o = pair_state[parity]
                    # Stage AV to SBUF right away so the PSUM bank frees fast.
                    o_raw = orp.tile([HD + 1, SG], f32, name="o_raw",
                                     tag="o_raw")
                    nc.vector.tensor_copy(o_raw, psum_o)
                    # Round-trip denominators through DRAM to spread them over
                    # 128 lanes (fast reciprocal), broadcast back via a
                    # partition-step-0 DRAM read. DMA latency only.
                    d1 = drp.tile([1, SG], f32, name="d1", tag="d1")
                    nc.sync.dma_start(out=d1, in_=o_raw[HD:HD + 1, :])
                    if lnexp:
                        # Low-latency variant (the very last chain): broadcast
                        # the raw denominators; reciprocal happens on the (by
                        # then idle) scalar engine as exp(-ln(x)).
                        r_raw = rp.tile([HD, SG], f32, name="r_raw",
                                        tag="r_raw")
                        nc.sync.dma_start(
                            out=r_raw,
                            in_=bass.AP(tensor=d1.tensor, offset=d1.offset,
                                        ap=[[0, HD]] + [list(p) for p in d1.ap[1:]]))
                        handoff[parity] = [o_raw, r_raw]
                        return
                    den_t = recp.tile([128, SG // 128], f32, name="den_t",
                                      tag="den_t")
                    nc.sync.dma_start(
                        out=den_t,
                        in_=d1.rearrange("a (p c) -> (a p) c", p=128))
                    handoff[parity] = [o_raw, den_t]
                return normA

            def make_normB(ct, g, handoff, parity, lnexp=False):
                def normB():
                    o_raw, den_t = handoff[parity]
                    if lnexp:
                        lnv = rp.tile([HD, SG], f32, name="lnv", tag="lnv")
                        nc.scalar.activation(
                            lnv, den_t, mybir.ActivationFunctionType.Ln)
                        r_sb = rp.tile([HD, SG], f32, name="r_sb", tag="r_sb")
                        nc.scalar.activation(
                            r_sb, lnv, Exp, scale=-1.0)
                        handoff[parity] = [o_raw, r_sb]
                        return
                    rec_t = recp.tile([128, SG // 128], f32, name="rec_t",
                                      tag="rec_t")
                    nc.vector.reciprocal(rec_t, den_t)
                    d2 = drp.tile([1, SG], f32, name="d2", tag="d2")
                    nc.sync.dma_start(
                        out=d2.rearrange("a (p c) -> (a p) c", p=128),
                        in_=rec_t)
                    r_sb = rp.tile([HD, SG], f32, name="r_sb", tag="r_sb")
                    nc.sync.dma_start(
                        out=r_sb,
                        in_=bass.AP(tensor=d2.tensor, offset=d2.offset,
                                    ap=[[0, HD]] + [list(p) for p in d2.ap[1:]]))
                    handoff[parity] = [o_raw, r_sb]
                return normB

            def make_normC(ct, g, handoff, parity):
                def normC():
                    po_sl = slice(64 * parity, 64 * parity + 64)
                    q_sl = slice(g * SG, (g + 1) * SG)
                    o_raw, r_sb = handoff[parity]
                    nc.vector.tensor_mul(oT[po_sl, ct, q_sl], o_raw[0:HD, :],
                                         r_sb)
                return normC

            # ---- schedule -------------------------------------------------
            # Warm the PE array (and flip the HAM clock gate to 8/8) with
            # dummy matmuls that run during the initial DMA wait, so the
            # first real matmuls start at full clock.
            warm = consts.tile([128, SG], bf16, name="warm")
            nc.vector.memset(warm, 0.5)
            pw = ps_f.tile([128, SG], f32, name="pw", tag="ps_f")
            for _ in range(25):
                nc.tensor.matmul(pw, lhsT=warm[:, 0:128], rhs=warm,
                                 start=True, stop=True)

            # DMA order matters at startup: the first QKV matmuls need only
            # xT(group 0) + wq, so those descriptors must head the queues.
            # qkv_gen's first chunk (before its first yield) is the xT DMA;
            # the generator body only touches w*_sb tiles after later yields,
            # by which time the load_w calls below have run. wp is not needed
            # until proj(0) runs (group 2), so it loads after the prologue.
            qkv0 = qkv_gen(0)
            next(qkv0)                       # emits xT(0) DMA first

            wq_sb = load_w(wq_d, [128, 8, LCOL], "wq_sb")
            tri_sb = consts.tile([128, 2, 128], bf16)
            nc.sync.dma_start(out=tri_sb, in_=tri_d.ap())
            bq_sb = consts.tile([128, 4], f32)
            nc.sync.dma_start(out=bq_sb,
                              in_=bq_d.ap().rearrange("(c p) -> p c", p=128))
            bk_sb = consts.tile([128, 4], f32)
            nc.sync.dma_start(out=bk_sb,
                              in_=bk_d.ap().rearrange("(c p) -> p c", p=128))
            wk_sb = load_w(wk_d, [128, 8, LCOL], "wk_sb")
            wv_sb = load_w(wv_d, [128, 8, LCOL], "wv_sb")

            # Prologue: q/k for pair 0 + v for kbs 0-3, dense.
            for _ in range(6):
                next(qkv0)

            wp_sb = load_w(wp_d, [128, 4, D], "wp_sb")

            defB, defC, defC_next = [], [], []
            queue = []
            for g in range(NSG):
                fill = []
                if g == 0:
                    fill.append(qkv0)        # remaining q/k pairs 1-3
                if g < NSG - 1:
                    fill.append(qkv_gen(g + 1))
                if g == 2:
                    fill.append(proj_gen(0))
                if g == 3:
                    fill.append(proj_gen(1))
                    fill.append(proj_gen(2, s4s=(0, 1)))
                # Strides underfeed on purpose: leftover chunks drain at the
                # group boundary, keeping the PE busy (and HAM warm) through
                # the exp/normalize drain of the last head-pair.
                stride = {0: 1, 1: 2, 2: 2, 3: 6}[g]
                state = {"i": 0}

                def tick():
                    state["i"] += 1
                    if state["i"] % stride == 0 and fill:
                        try:
                            next(fill[0])
                        except StopIteration:
                            fill.pop(0)

                for ct in range(4):
                    pair_state = {}
                    handoff = {}
                    for kb in range(4 * g + 2):
                        attn_unit(ct, g, kb, pair_state, queue, tick)
                    attn_unit_m(ct, g, pair_state, queue, tick)
                    queue.append(make_normA(ct, g, pair_state, 0, handoff))
                    queue.append(make_normA(ct, g, pair_state, 1, handoff))
                    queue.extend(defB)       # pair ct-1: reciprocal + spread
                    queue.extend(defC)       # pair ct-2: oT multiply
                    defC = defC_next
                    defC_next = [make_normC(ct, g, handoff, 0),
                                 make_normC(ct, g, handoff, 1)]
                    defB = [make_normB(ct, g, handoff, 0),
                            make_normB(ct, g, handoff, 1)]
                # Group boundary: the exp/AV queue is carried into the next
                # group (its entries pop during the next group's first units,
                # keeping the PE pipeline full through the boundary). Only
                # fill must drain here so producer matmuls stay ahead of the
                # next group's S units in the tensor FIFO.
                for gen in fill:
                    for _ in gen:
                        pass
            while queue:         # end of last group
                queue.pop(0)()
            # Final projection, overlapped with the last normalize chains:
            # seq-blocks 12-13 accumulate their c=0..2 partials in borrowed
            # ps_s/ps_o slots (free once the last ACT / o_raw copies ran) so
            # only the c=3 matmul + evacuation remains after the last oT
            # multiply. Seq-blocks 14-15 run the normal full chains.
            for fn in defC:      # oT multiply of pair ct2 (its B already ran)
                fn()
            # proj(2)'s second half was held out of the g3 fill so its
            # matmuls land here (must precede the held ps_f partials below
            # so the ps_f slot ordering doesn't serialize it behind them).
            for _ in proj_gen(2, s4s=(2, 3)):
                pass
            # Final projection seq-blocks 12-14 accumulate their c=0..2
            # partials in borrowed/held psum slots so only the c=3 matmul +
            # evacuation remains after the last oT multiply.
            part = []
            for u, (s4, j) in enumerate(((0, 0), (0, 1), (1, 0), (1, 1),
                                         (2, 0), (2, 1))):
                sb = 4 * (NSG - 1) + s4
                pool, tag = ((ps_s, "ps_s") if u < 2 else
                             (ps_o, "ps_o") if u < 4 else (ps_f, "ps_f"))
                pj = pool.tile([128, SG], f32, name="pj", tag=tag)
                for c in range(3):
                    nc.tensor.matmul(
                        pj, lhsT=oT[:, c, 128 * sb:128 * (sb + 1)],
                        rhs=wp_sb[:, c, j * SG:(j + 1) * SG],
                        start=(c == 0), stop=False)
                part.append((sb, j, pj))
            for fn in defB + defC_next:          # last pair: B then oT mul
                fn()
            # The c=3 contraction is split into K=64 halves: the even-head
            # half only needs the even-head oT multiply (which lands ~1.4us
            # before the odd one), so these six matmuls start early and the
            # halves row-tile on the PE array.
            for sb, j, pj in part:
                nc.tensor.matmul(
                    pj, lhsT=oT[0:64, 3, 128 * sb:128 * (sb + 1)],
                    rhs=wp_sb[0:64, 3, j * SG:(j + 1) * SG],
                    start=False, stop=False)
            o_sbs = {}
            for sb, j, pj in part:
                nc.tensor.matmul(
                    pj, lhsT=oT[64:128, 3, 128 * sb:128 * (sb + 1)],
                    rhs=wp_sb[64:128, 3, j * SG:(j + 1) * SG],
                    start=False, stop=True)
                if sb not in o_sbs:
                    o_sbs[sb] = outp.tile([128, 2, SG], bf16, name="o_sb",
                                          tag="o_sb")
                nc.vector.tensor_copy(o_sbs[sb][:, j, :], pj)
            for sb, o_sb in o_sbs.items():
                nc.sync.dma_start(
                    out=out_d.ap()[128 * sb:128 * (sb + 1), :]
                    .rearrange("p (j n) -> p j n", j=2),
                    in_=o_sb)
            for _ in proj_gen(NSG - 1, s4s=(3,)):
                pass

            if debug_dump:
                for nm, t in (("qT", qT), ("kT", kT), ("v_ext", v_ext),
                              ("oT", oT)):
                    dmp = nc.dram_tensor(f"dump_{nm}", list(t.shape), bf16,
                                         kind="ExternalOutput")
                    nc.sync.dma_start(out=dmp.ap(), in_=t)

    nc.compile()
    return nc


def _get_nc():
    if "nc" not in _CACHE:
        _CACHE["nc"] = _build()
    return _CACHE["nc"]


def _make_tri():
    """tri[kl, :, c] = 1.0 iff kl <= c (bf16), for 128-wide diagonal blocks,
    duplicated on axis 1 so one DVE multiply covers both heads of a pair."""
    import ml_dtypes
    kl = np.arange(128)[:, None]
    c = np.arange(128)[None, :]
    t = (kl <= c).astype(ml_dtypes.bfloat16)
    return np.ascontiguousarray(np.broadcast_to(t[:, None, :], (128, 2, 128)))


def make_in_maps(x, Wq, bq, Wk, bk, Wv, Wp):
    import ml_dtypes
    bf = ml_dtypes.bfloat16
    tri = _make_tri()
    xt = {}
    wmaps = {}

    def tile_w(w2d, chunks):
        # [128*chunks, n] -> [128, chunks, n] with 128c+p row mapping
        n = w2d.shape[1]
        return np.ascontiguousarray(
            w2d.reshape(chunks, 128, n).transpose(1, 0, 2).astype(bf))

    for hg in range(2):
        hs = slice(hg * HPC, (hg + 1) * HPC)
        wmaps[hg] = {
            "wq": tile_w(Wq[hs].transpose(1, 0, 2).reshape(D, LCOL), 8),
            "wk": tile_w(Wk[hs].transpose(1, 0, 2).reshape(D, LCOL), 8),
            "wv": tile_w(Wv[hs].transpose(1, 0, 2).reshape(D, LCOL), 8),
            "wp": tile_w(Wp[hg * LCOL:(hg + 1) * LCOL, :], 4),
            "bq": np.ascontiguousarray(bq[hs].reshape(LCOL)).astype(np.float32),
            "bk": np.ascontiguousarray(bk[hs].reshape(LCOL)).astype(np.float32),
        }
    in_maps = []
    for c in range(8):
        b, hg = c // 2, c % 2
        if b not in xt:
            # xtg[g, p, c, s] = x[b][512g+s, 128c+p]
            xt[b] = np.ascontiguousarray(
                np.asarray(x[b]).reshape(NSG, SG, 8, 128)
                .transpose(0, 3, 2, 1).astype(bf))
        in_maps.append({"xtg": xt[b], "tri": tri, **wmaps[hg]})
    return in_maps


def combine(results, Wp, bv, bp):
    """Unshard: sum the two head-group partials per batch + linear bias terms."""
    add = bp + bv.reshape(D) @ Wp
    out = np.empty((B, S, D), np.float32)
    for b in range(B):
        out[b] = (results[2 * b]["out"].astype(np.float32)
                  + results[2 * b + 1]["out"].astype(np.float32) + add)
    return out


def kernel(x, Wq, bq, Wk, bk, Wv, bv, Wp, bp):
    from concourse.bass_utils import run_bass_kernel_spmd

    x = np.asarray(x, np.float32)
    Wq = np.asarray(Wq, np.float32)
    Wk = np.asarray(Wk, np.float32)
    Wv = np.asarray(Wv, np.float32)
    bq = np.asarray(bq, np.float32)
    bk = np.asarray(bk, np.float32)
    bv = np.asarray(bv, np.float32)
    Wp = np.asarray(Wp, np.float32)
    bp = np.asarray(bp, np.float32)

    nc = _get_nc()
    in_maps = make_in_maps(x, Wq, bq, Wk, bk, Wv, Wp)
    res = run_bass_kernel_spmd(nc, in_maps, core_ids=list(range(8)))
    return combine(res.results, Wp, bv, bp)


# revision 47
# speedup vs baseline: 1.0134x; 1.0134x over previous
"""Multi-head causal attention (B=4, S=2048, D=1024, H=16) on 8 TRN2 cores.

Sharding: data-parallel over batch (4) x tensor-parallel over heads (2 groups
of 8 heads). Core c handles batch c//2, head-group c%2. Each core computes
q/k/v projections for its 8 heads, causal flash-style attention, and a partial
output projection against its row-shard of Wp. Host sums the two partials per
batch and adds the bias terms (bp + bv @ Wp, which commute with the row-sum).

Key layout/scheduling choices (v3):
- x arrives pre-transposed, pre-tiled and pre-cast to bf16 from the host, as do
  all weights: DMA descriptors are 4-8KB contiguous runs (descriptor-rate is
  the startup bottleneck), and there are no on-chip transposes or casts.
- Scores are computed as S^T = kT^T qT with keys on partitions; the softmax
  denominator falls out of the AV matmul via a ones-column appended to V.
- Heads are processed in (even, odd) pairs living on partitions 0-63 / 64-127.
  The two K=64 S-matmuls of a pair are emitted back-to-back so the PE array
  row-tiles them (tile_position (0,0) / (64,0)) and runs them concurrently.
  One unit = (pair, key-block): S psum is [128, 2(parity), 512] so a single
  exp ACT covers both heads of the pair (amortizes the ~290ns ACT overhead).
- Diagonal 128-key blocks only stream the causally-live query range (q0 =
  128*(kb-4g)); exp is likewise restricted, and one [128, 2, 128] lower-tri
  mask handles the partial block. Fully masked regions are never written/read.
- exp/AV emission lags S emission by one unit (shared queue) so the scalar
  engine's exp pipelines with the tensor engine; qkv/proj work for other
  groups is woven into the exp-bound attention stretches as fill.
- Softmax denominators round-trip through DRAM for a 128-lane reciprocal; the
  final oT multiply is deferred by one head-pair so the DMA latency never
  blocks the (strict-FIFO) vector engine queue.
"""

import numpy as np

B, S, D, H = 4, 2048, 1024, 16
HD = D // H            # head_size = 64
HPC = 8                # heads per core
LCOL = HPC * HD        # 512 local columns
NSG = 4                # seq groups of 512
SG = S // NSG          # 512
NKB = S // 128         # 16 key blocks of 128

_CACHE = {}


def _build(debug_dump=False):
    import concourse.bass as bass
    import concourse.tile as tile
    from concourse import bacc, mybir

    f32 = mybir.dt.float32
    bf16 = mybir.dt.bfloat16

    nc = bacc.Bacc("TRN2", target_bir_lowering=False, debug=False)

    xtg_d = nc.dram_tensor("xtg", [NSG, 128, 8, SG], bf16, kind="ExternalInput")
    wq_d = nc.dram_tensor("wq", [128, 8, LCOL], bf16, kind="ExternalInput")
    wk_d = nc.dram_tensor("wk", [128, 8, LCOL], bf16, kind="ExternalInput")
    wv_d = nc.dram_tensor("wv", [128, 8, LCOL], bf16, kind="ExternalInput")
    wp_d = nc.dram_tensor("wp", [128, 4, D], bf16, kind="ExternalInput")
    bq_d = nc.dram_tensor("bq", [LCOL], f32, kind="ExternalInput")
    bk_d = nc.dram_tensor("bk", [LCOL], f32, kind="ExternalInput")
    tri_d = nc.dram_tensor("tri", [128, 2, 128], bf16, kind="ExternalInput")
    out_d = nc.dram_tensor("out", [S, D], bf16, kind="ExternalOutput")

    Exp = mybir.ActivationFunctionType.Exp

    with tile.TileContext(nc) as tc:
        with (
            tc.tile_pool(name="consts", bufs=1) as consts,
            tc.tile_pool(name="xtp", bufs=2) as xtp,
            tc.tile_pool(name="acts", bufs=1) as acts,
            tc.tile_pool(name="pp", bufs=6) as pp,
            tc.tile_pool(name="recp", bufs=6) as recp,
            tc.tile_pool(name="orp", bufs=8) as orp,
            tc.tile_pool(name="rp", bufs=6) as rp,
            tc.tile_pool(name="outp", bufs=4) as outp,
            tc.tile_pool(name="drp", bufs=2, space="DRAM") as drp,
            tc.tile_pool(name="ps_s", bufs=2, space="PSUM") as ps_s,
            tc.tile_pool(name="ps_o", bufs=2, space="PSUM") as ps_o,
            tc.tile_pool(name="ps_f", bufs=2, space="PSUM") as ps_f,
        ):
            # ---- weights (host already bf16 + pre-tiled; 2 DMAs each) -----
            def load_w(dram, shape, name, nsp=2):
                t = consts.tile(shape, bf16, name=name)
                w = shape[1] // nsp
                for i in range(nsp):
                    nc.sync.dma_start(out=t[:, i * w:(i + 1) * w, :],
                                      in_=dram.ap()[:, i * w:(i + 1) * w, :])
                return t



            # ---- persistent activations ----------------------------------
            qT = acts.tile([128, 4, S], bf16)      # [head-dim%128, pair, seq]
            kT = acts.tile([128, 4, S], bf16)
            oT = acts.tile([128, 4, S], bf16)
            v_ext = acts.tile([128, NKB, HPC, HD + 1], bf16)
            for h in range(HPC):                   # ones columns (denominator)
                nc.vector.memset(v_ext[:, :, h, HD:HD + 1], 1.0)

            def qkv_gen(g):
                """q/k/v projections for seq group g. Chunk order: q/k for
                pair 0, then v (all kbs), then q/k for pairs 1-3 so the
                attention of (g, pair 0) can start as early as possible."""
                xT = xtp.tile([128, 8, SG], bf16, name="xT", tag="xT")
                nc.sync.dma_start(out=xT[:, 0:4, :], in_=xtg_d.ap()[g][:, 0:4, :])
                nc.sync.dma_start(out=xT[:, 4:8, :], in_=xtg_d.ap()[g][:, 4:8, :])
                yield

                def qk_m(m):
                    for w_sb, b_sb, dstT in ((wq_sb, bq_sb, qT),
                                             (wk_sb, bk_sb, kT)):
                        pq = ps_f.tile([128, SG], f32, name="pq", tag="ps_f")
                        for dc in range(8):
                            nc.tensor.matmul(
                                pq, lhsT=w_sb[:, dc, 128 * m:128 * (m + 1)],
                                rhs=xT[:, dc, :], start=(dc == 0),
                                stop=(dc == 7))
                        nc.vector.tensor_scalar_add(
                            dstT[:, m, g * SG:(g + 1) * SG], pq,
                            b_sb[:, m:m + 1])
                        yield

                yield from qk_m(0)
                for s4 in range(4):
                    pv = ps_f.tile([128, LCOL], f32, name="pv", tag="ps_f")
                    for dc in range(8):
                        nc.tensor.matmul(
                            pv, lhsT=xT[:, dc, 128 * s4:128 * (s4 + 1)],
                            rhs=wv_sb[:, dc, :], start=(dc == 0), stop=(dc == 7))
                    kb = 4 * g + s4
                    nc.vector.tensor_copy(
                        v_ext[:, kb, :, 0:HD],
                        pv.rearrange("p (h e) -> p h e", e=HD))
                    yield
                for m in range(1, 4):
                    yield from qk_m(m)

            def proj_gen(g, s4s=(0, 1, 2, 3)):
                for s4 in s4s:
                    sb = 4 * g + s4
                    o_sb = outp.tile([128, 2, SG], bf16, name="o_sb", tag="o_sb")
                    for j in range(2):
                        ppr = ps_f.tile([128, SG], f32, name="ppr", tag="ps_f")
                        for c in range(4):
                            nc.tensor.matmul(
                                ppr, lhsT=oT[:, c, 128 * sb:128 * (sb + 1)],
                                rhs=wp_sb[:, c, j * SG:(j + 1) * SG],
                                start=(c == 0), stop=(c == 3))
                        nc.vector.tensor_copy(o_sb[:, j, :], ppr)
                        yield
                    nc.sync.dma_start(
                        out=out_d.ap()[128 * sb:128 * (sb + 1), :]
                        .rearrange("p (j n) -> p j n", j=2),
                        in_=o_sb)

            def attn_unit(ct, g, kb, pair_state, queue, tick):
                """One (head-pair, key-block) unit: emits the two row-tiled
                S matmuls now; queues exp+mask+AV for later."""
                nkb = 4 * g + 4
                q0 = max(0, 128 * kb - g * SG)
                pst = ps_s.tile([128, 2, SG], f32, name="pst", tag="ps_s")
                for parity in (0, 1):
                    po = slice(64 * parity, 64 * parity + 64)
                    nc.tensor.matmul(
                        pst[:, parity, q0:SG],
                        lhsT=kT[po, ct, 128 * kb:128 * (kb + 1)],
                        rhs=qT[po, ct, g * SG + q0:(g + 1) * SG],
                        start=True, stop=True)
                tick()

                def exp_av():
                    if kb == 0:
                        pair_state[0] = ps_o.tile([HD + 1, SG], f32,
                                                  name="po0", tag="ps_o")
                        pair_state[1] = ps_o.tile([HD + 1, SG], f32,
                                                  name="po1", tag="ps_o")
                    p_sb = pp.tile([128, 2, SG], bf16, name="p_sb", tag="p_sb")
                    nc.scalar.activation(p_sb[:, :, q0:SG], pst[:, :, q0:SG],
                                         Exp, scale=0.125)
                    if kb >= 4 * g:    # diagonal block: lower-tri mask
                        nc.vector.tensor_mul(
                            p_sb[:, :, q0:q0 + 128], p_sb[:, :, q0:q0 + 128],
                            tri_sb)
                    for parity in (0, 1):
                        h = 2 * ct + parity
                        nc.tensor.matmul(
                            pair_state[parity][:, q0:SG],
                            lhsT=v_ext[:, kb, h, :],
                            rhs=p_sb[:, parity, q0:SG],
                            start=(kb == 0), stop=(kb == nkb - 1))

                queue.append(exp_av)
                while len(queue) > 1:
                    queue.pop(0)()

            def attn_unit_m(ct, g, pair_state, queue, tick):
                """Merged unit for the last two diagonal key-blocks: their
                causally-live query ranges (256 + 128 cols) share one PSUM
                bank pair, so a single exp ACT covers both (saves ~290ns of
                ACT overhead per pair). kb_a's range q[256:512) is stored at
                cols [0:256); kb_b's q[384:512) at cols [256:384)."""
                nkb = 4 * g + 4
                kb_a, kb_b = 4 * g + 2, 4 * g + 3
                pst = ps_s.tile([128, 2, SG], f32, name="pst", tag="ps_s")
                for kb, c0, q0 in ((kb_a, 0, 256), (kb_b, 256, 384)):
                    n = SG - q0
                    for parity in (0, 1):
                        po = slice(64 * parity, 64 * parity + 64)
                        nc.tensor.matmul(
                            pst[:, parity, c0:c0 + n],
                            lhsT=kT[po, ct, 128 * kb:128 * (kb + 1)],
                            rhs=qT[po, ct, g * SG + q0:(g + 1) * SG],
                            start=True, stop=True)
                tick()

                def exp_av():
                    p_sb = pp.tile([128, 2, SG], bf16, name="p_sb", tag="p_sb")
                    nc.scalar.activation(p_sb[:, :, 0:384], pst[:, :, 0:384],
                                         Exp, scale=0.125)
                    nc.vector.tensor_mul(p_sb[:, :, 0:128],
                                         p_sb[:, :, 0:128], tri_sb)
                    nc.vector.tensor_mul(p_sb[:, :, 256:384],
                                         p_sb[:, :, 256:384], tri_sb)
                    for parity in (0, 1):
                        h = 2 * ct + parity
                        nc.tensor.matmul(
                            pair_state[parity][:, 256:SG],
                            lhsT=v_ext[:, kb_a, h, :],
                            rhs=p_sb[:, parity, 0:256],
                            start=False, stop=False)
                        nc.tensor.matmul(
                            pair_state[parity][:, 384:SG],
                            lhsT=v_ext[:, kb_b, h, :],
                            rhs=p_sb[:, parity, 256:384],
                            start=False, stop=(kb_b == nkb - 1))

                queue.append(exp_av)
                while len(queue) > 1:
                    queue.pop(0)()

            # Normalization runs as a 3-stage pipeline, each stage deferred
            # by one head-pair so no DVE op ever waits on a DMA round-trip
            # at the head of the (strict FIFO) vector queue.
            def make_normA(ct, g, pair_state, parity, handoff, lnexp=False):
                def normA():
                    psum_o = pair_state[parity]
                    # Stage AV to SBUF right away so the PSUM bank frees fast.
                    o_raw = orp.tile([HD + 1, SG], f32, name="o_raw",
                                     tag="o_raw")
                    nc.vector.tensor_copy(o_raw, psum_o)
                    # Round-trip denominators through DRAM to spread them over
                    # 128 lanes (fast reciprocal), broadcast back via a
                    # partition-step-0 DRAM read. DMA latency only.
                    d1 = drp.tile([1, SG], f32, name="d1", tag="d1")
                    nc.sync.dma_start(out=d1, in_=o_raw[HD:HD + 1, :])
                    if lnexp:
                        # Low-latency variant (the very last chain): broadcast
                        # the raw denominators; reciprocal happens on the (by
                        # then idle) scalar engine as exp(-ln(x)).
                        r_raw = rp.tile([HD, SG], f32, name="r_raw",
                                        tag="r_raw")
                        nc.sync.dma_start(
                            out=r_raw,
                            in_=bass.AP(tensor=d1.tensor, offset=d1.offset,
                                        ap=[[0, HD]] + [list(p) for p in d1.ap[1:]]))
                        handoff[parity] = [o_raw, r_raw]
                        return
                    den_t = recp.tile([128, SG // 128], f32, name="den_t",
                                      tag="den_t")
                    nc.sync.dma_start(
                        out=den_t,
                        in_=d1.rearrange("a (p c) -> (a p) c", p=128))
                    handoff[parity] = [o_raw, den_t]
                return normA

            def make_normB(ct, g, handoff, parity, lnexp=False):
                def normB():
                    o_raw, den_t = handoff[parity]
                    if lnexp:
                        lnv = rp.tile([HD, SG], f32, name="lnv", tag="lnv")
                        nc.scalar.activation(
                            lnv, den_t, mybir.ActivationFunctionType.Ln)
                        r_sb = rp.tile([HD, SG], f32, name="r_sb", tag="r_sb")
                        nc.scalar.activation(
                            r_sb, lnv, Exp, scale=-1.0)
                        handoff[parity] = [o_raw, r_sb]
                        return
                    rec_t = recp.tile([128, SG // 128], f32, name="rec_t",
                                      tag="rec_t")
                    nc.vector.reciprocal(rec_t, den_t)
                    d2 = drp.tile([1, SG], f32, name="d2", tag="d2")
                    nc.sync.dma_start(
                        out=d2.rearrange("a (p c) -> (a p) c", p=128),
                        in_=rec_t)
                    r_sb = rp.tile([HD, SG], f32, name="r_sb", tag="r_sb")
                    nc.sync.dma_start(
                        out=r_sb,
                        in_=bass.AP(tensor=d2.tensor, offset=d2.offset,
                                    ap=[[0, HD]] + [list(p) for p in d2.ap[1:]]))
                    handoff[parity] = [o_raw, r_sb]
                return normB

            def make_normC(ct, g, handoff, parity):
                def normC():
                    po_sl = slice(64 * parity, 64 * parity + 64)
                    q_sl = slice(g * SG, (g + 1) * SG)
                    o_raw, r_sb = handoff[parity]
                    nc.vector.tensor_mul(oT[po_sl, ct, q_sl], o_raw[0:HD, :],
                                         r_sb)
                return normC

            # ---- schedule -------------------------------------------------
            # Warm the PE array (and flip the HAM clock gate to 8/8) with
            # dummy matmuls that run during the initial DMA wait, so the
            # first real matmuls start at full clock.
            warm = consts.tile([128, SG], bf16, name="warm")
            nc.vector.memset(warm, 0.5)
            pw = ps_f.tile([128, SG], f32, name="pw", tag="ps_f")
            for _ in range(25):
                nc.tensor.matmul(pw, lhsT=warm[:, 0:128], rhs=warm,
                                 start=True, stop=True)

            # DMA order matters at startup: the first QKV matmuls need only
            # xT(group 0) + wq, so those descriptors must head the queues.
            # qkv_gen's first chunk (before its first yield) is the xT DMA;
            # the generator body only touches w*_sb tiles after later yields,
            # by which time the load_w calls below have run. wp is not needed
            # until proj(0) runs (group 2), so it loads after the prologue.
            qkv0 = qkv_gen(0)
            next(qkv0)                       # emits xT(0) DMA first

            wq_sb = load_w(wq_d, [128, 8, LCOL], "wq_sb")
            tri_sb = consts.tile([128, 2, 128], bf16)
            nc.sync.dma_start(out=tri_sb, in_=tri_d.ap())
            bq_sb = consts.tile([128, 4], f32)
            nc.sync.dma_start(out=bq_sb,
                              in_=bq_d.ap().rearrange("(c p) -> p c", p=128))
            bk_sb = consts.tile([128, 4], f32)
            nc.sync.dma_start(out=bk_sb,
                              in_=bk_d.ap().rearrange("(c p) -> p c", p=128))
            wk_sb = load_w(wk_d, [128, 8, LCOL], "wk_sb")
            wv_sb = load_w(wv_d, [128, 8, LCOL], "wv_sb")

            # Prologue: q/k for pair 0 + v for kbs 0-3, dense.
            for _ in range(6):
                next(qkv0)

            wp_sb = load_w(wp_d, [128, 4, D], "wp_sb")

            defB, defC, defC_next = [], [], []
            queue = []
            for g in range(NSG):
                fill = []
                if g == 0:
                    fill.append(qkv0)        # remaining q/k pairs 1-3
                if g < NSG - 1:
                    fill.append(qkv_gen(g + 1))
                if g == 2:
                    fill.append(proj_gen(0))
                if g == 3:
                    fill.append(proj_gen(1))
                    fill.append(proj_gen(2, s4s=(0, 1)))
                # Strides underfeed on purpose: leftover chunks drain at the
                # group boundary, keeping the PE busy (and HAM warm) through
                # the exp/normalize drain of the last head-pair.
                stride = {0: 1, 1: 2, 2: 2, 3: 6}[g]
                state = {"i": 0}

                def tick():
                    state["i"] += 1
                    if state["i"] % stride == 0 and fill:
                        try:
                            next(fill[0])
                        except StopIteration:
                            fill.pop(0)

                for ct in range(4):
                    pair_state = {}
                    handoff = {}
                    for kb in range(4 * g + 2):
                        attn_unit(ct, g, kb, pair_state, queue, tick)
                    attn_unit_m(ct, g, pair_state, queue, tick)
                    queue.append(make_normA(ct, g, pair_state, 0, handoff))
                    queue.append(make_normA(ct, g, pair_state, 1, handoff))
                    queue.extend(defB)       # pair ct-1: reciprocal + spread
                    queue.extend(defC)       # pair ct-2: oT multiply
                    defC = defC_next
                    defC_next = [make_normC(ct, g, handoff, 0),
                                 make_normC(ct, g, handoff, 1)]
                    defB = [make_normB(ct, g, handoff, 0),
                            make_normB(ct, g, handoff, 1)]
                # Group boundary: the exp/AV queue is carried into the next
                # group (its entries pop during the next group's first units,
                # keeping the PE pipeline full through the boundary). Only
                # fill must drain here so producer matmuls stay ahead of the
                # next group's S units in the tensor FIFO.
                for gen in fill:
                    for _ in gen:
                        pass
            while queue:         # end of last group
                queue.pop(0)()
            # Final projection, overlapped with the last normalize chains:
            # seq-blocks 12-13 accumulate their c=0..2 partials in borrowed
            # ps_s/ps_o slots (free once the last ACT / o_raw copies ran) so
            # only the c=3 matmul + evacuation remains after the last oT
            # multiply. Seq-blocks 14-15 run the normal full chains.
            for fn in defC:      # oT multiply of pair ct2 (its B already ran)
                fn()
            # proj(2)'s second half was held out of the g3 fill so its
            # matmuls land here (must precede the held ps_f partials below
            # so the ps_f slot ordering doesn't serialize it behind them).
            for _ in proj_gen(2, s4s=(2, 3)):
                pass
            # Final projection seq-blocks 12-14 accumulate their c=0..2
            # partials in borrowed/held psum slots so only the c=3 matmul +
            # evacuation remains after the last oT multiply.
            part = []
            for u, (s4, j) in enumerate(((0, 0), (0, 1), (1, 0), (1, 1),
                                         (2, 0), (2, 1))):
                sb = 4 * (NSG - 1) + s4
                pool, tag = ((ps_s, "ps_s") if u < 2 else
                             (ps_o, "ps_o") if u < 4 else (ps_f, "ps_f"))
                pj = pool.tile([128, SG], f32, name="pj", tag=tag)
                for c in range(3):
                    nc.tensor.matmul(
                        pj, lhsT=oT[:, c, 128 * sb:128 * (sb + 1)],
                        rhs=wp_sb[:, c, j * SG:(j + 1) * SG],
                        start=(c == 0), stop=False)
                part.append((sb, j, pj))
            for fn in defB + defC_next:          # last pair: B then oT mul
                fn()
            o_sbs = {}
            for sb, j, pj in part:
                nc.tensor.matmul(
                    pj, lhsT=oT[:, 3, 128 * sb:128 * (sb + 1)],
                    rhs=wp_sb[:, 3, j * SG:(j + 1) * SG],
                    start=False, stop=True)
                if sb not in o_sbs:
                    o_sbs[sb] = outp.tile([128, 2, SG], bf16, name="o_sb",
                                          tag="o_sb")
                nc.vector.tensor_copy(o_sbs[sb][:, j, :], pj)
            for sb, o_sb in o_sbs.items():
                nc.sync.dma_start(
                    out=out_d.ap()[128 * sb:128 * (sb + 1), :]
                    .rearrange("p (j n) -> p j n", j=2),
                    in_=o_sb)
            for _ in proj_gen(NSG - 1, s4s=(3,)):
                pass

            if debug_dump:
                for nm, t in (("qT", qT), ("kT", kT), ("v_ext", v_ext),
                              ("oT", oT)):
                    dmp = nc.dram_tensor(f"dump_{nm}", list(t.shape), bf16,
                                         kind="ExternalOutput")
                    nc.sync.dma_start(out=dmp.ap(), in_=t)

    nc.compile()
    return nc


def _get_nc():
    if "nc" not in _CACHE:
        _CACHE["nc"] = _build()
    return _CACHE["nc"]


def _make_tri():
    """tri[kl, :, c] = 1.0 iff kl <= c (bf16), for 128-wide diagonal blocks,
    duplicated on axis 1 so one DVE multiply covers both heads of a pair."""
    import ml_dtypes
    kl = np.arange(128)[:, None]
    c = np.arange(128)[None, :]
    t = (kl <= c).astype(ml_dtypes.bfloat16)
    return np.ascontiguousarray(np.broadcast_to(t[:, None, :], (128, 2, 128)))


def make_in_maps(x, Wq, bq, Wk, bk, Wv, Wp):
    import ml_dtypes
    bf = ml_dtypes.bfloat16
    tri = _make_tri()
    xt = {}
    wmaps = {}

    def tile_w(w2d, chunks):
        # [128*chunks, n] -> [128, chunks, n] with 128c+p row mapping
        n = w2d.shape[1]
        return np.ascontiguousarray(
            w2d.reshape(chunks, 128, n).transpose(1, 0, 2).astype(bf))

    for hg in range(2):
        hs = slice(hg * HPC, (hg + 1) * HPC)
        wmaps[hg] = {
            "wq": tile_w(Wq[hs].transpose(1, 0, 2).reshape(D, LCOL), 8),
            "wk": tile_w(Wk[hs].transpose(1, 0, 2).reshape(D, LCOL), 8),
            "wv": tile_w(Wv[hs].transpose(1, 0, 2).reshape(D, LCOL), 8),
            "wp": tile_w(Wp[hg * LCOL:(hg + 1) * LCOL, :], 4),
            "bq": np.ascontiguousarray(bq[hs].reshape(LCOL)).astype(np.float32),
            "bk": np.ascontiguousarray(bk[hs].reshape(LCOL)).astype(np.float32),
        }
    in_maps = []
    for c in range(8):
        b, hg = c // 2, c % 2
        if b not in xt:
            # xtg[g, p, c, s] = x[b][512g+s, 128c+p]
            xt[b] = np.ascontiguousarray(
                np.asarray(x[b]).reshape(NSG, SG, 8, 128)
                .transpose(0, 3, 2, 1).astype(bf))
        in_maps.append({"xtg": xt[b], "tri": tri, **wmaps[hg]})
    return in_maps


def combine(results, Wp, bv, bp):
    """Unshard: sum the two head-group partials per batch + linear bias terms."""
    add = bp + bv.reshape(D) @ Wp
    out = np.empty((B, S, D), np.float32)
    for b in range(B):
        out[b] = (results[2 * b]["out"].astype(np.float32)
                  + results[2 * b + 1]["out"].astype(np.float32) + add)
    return out


def kernel(x, Wq, bq, Wk, bk, Wv, bv, Wp, bp):
    from concourse.bass_utils import run_bass_kernel_spmd

    x = np.asarray(x, np.float32)
    Wq = np.asarray(Wq, np.float32)
    Wk = np.asarray(Wk, np.float32)
    Wv = np.asarray(Wv, np.float32)
    bq = np.asarray(bq, np.float32)
    bk = np.asarray(bk, np.float32)
    bv = np.asarray(bv, np.float32)
    Wp = np.asarray(Wp, np.float32)
    bp = np.asarray(bp, np.float32)

    nc = _get_nc()
    in_maps = make_in_maps(x, Wq, bq, Wk, bk, Wv, Wp)
    res = run_bass_kernel_spmd(nc, in_maps, core_ids=list(range(8)))
    return combine(res.results, Wp, bv, bp)
